# revision 12
# baseline (speedup 1.0000x reference)
"""Multi-head attention forward on 8 Trainium2 NeuronCores.

Sharding: core c = 2*b + g handles batch b (of 4) and head-group g (8 of 16
heads). Each core computes its group's attention output projected through its
slice of w_proj (row-parallel); the host sums the two partial products per
batch and adds the bias terms.

Math notes (exact identities, not approximations):
  - the key bias b_k adds a per-query constant to every score row, which
    softmax ignores;
  - the value bias b_v passes through attention unchanged (attn rows sum to 1)
    so its projection b_v @ w_proj is folded into the host-side bias;
  - the 1/sqrt(64) score scale is folded into w_q / b_q (exact: power of two).

Compute dtype is float32r (TensorE reduced-precision fp32 path): ~bf16 matmul
speed at 512-wide moving operands, ~2e-4 error instead of bf16's ~4e-3.

Layouts: scores are computed transposed (ST = kT.T @ qT, [s_j, s_i]) so the
exp output PT feeds the PV matmul directly as the moving operand with no
P-transpose pass. V carries an appended ones column, so the PV output row 64
is the softmax denominator Z. Normalization (1/Z) happens on the PV psum via
ACT ln->exp + GPSIMD partition-broadcast. Head pairs (2t, 2t+1) live in the
partition halves {0..63} / {64..127} of shared qk tiles, so their K=64 score
matmuls land in different PE row groups and execute concurrently.
"""

import numpy as np

import concourse.bass as bass
import concourse.tile as tile
from concourse import bacc, mybir
from concourse import bass_utils

F32 = mybir.dt.float32
F32R = mybir.dt.float32r
AF = mybir.ActivationFunctionType

B, S, D = 4, 2048, 1024
H, HD = 16, 64
HG = 8            # heads per core (group)
N_CORES = 8

_CACHE = {}


def _build():
    nc = bacc.Bacc("TRN2", target_bir_lowering=False, debug=False,
                   num_devices=N_CORES)
    xt_d = nc.dram_tensor("xt", [D, S], F32R, kind="ExternalInput").ap()
    wqk_d = nc.dram_tensor("wqk", [D, 2 * HG * HD], F32R, kind="ExternalInput").ap()
    wv_d = nc.dram_tensor("wv", [D, HG * HD], F32R, kind="ExternalInput").ap()
    wp_d = nc.dram_tensor("wp", [HG * HD, D], F32R, kind="ExternalInput").ap()
    bq_d = nc.dram_tensor("bq", [128, 4], F32, kind="ExternalInput").ap()
    out_d = nc.dram_tensor("out", [S, D], F32, kind="ExternalOutput").ap()

    KT = D // 128          # 8 k-tiles over the embedding dim
    ST16 = S // 128        # 16 tiles over sequence

    with tile.TileContext(nc) as tc:
        with (
            tc.tile_pool(name="persist", bufs=1) as pp,
            tc.tile_pool(name="psum", bufs=1, space="PSUM") as ps,
        ):
            # ---- persistent SBUF tensors ----
            qk_sb = [pp.tile([128, S], F32R, name=f"qk{m}", tag=f"qk{m}")
                     for m in range(8)]
            v_sb = [pp.tile([128, HG, HD + 1], F32R, name=f"v{j}", tag=f"v{j}")
                    for j in range(ST16)]
            bq_sb = pp.tile([128, 4], F32, tag="bq")
            nc.sync.dma_start(bq_sb[:], bq_d)
            ones_sb = pp.tile([128, HG, 1], F32, tag="ones")
            nc.vector.memset(ones_sb[:], 1.0)

            # ============ phases A+B: qkT and v (xt fully resident) =========
            with tc.tile_pool(name="xtp", bufs=1) as xtp, \
                 tc.tile_pool(name="wabp", bufs=1) as wabp:
                xt_sb = xtp.tile([128, KT, S], F32R, tag="xt")
                for k in range(KT):
                    nc.sync.dma_start(xt_sb[:, k, :], xt_d[k * 128:(k + 1) * 128, :])
                wv_sb = [wabp.tile([128, 512], F32R, name=f"wv{k}", tag=f"wv{k}")
                         for k in range(KT)]
                for k in range(KT):
                    nc.sync.dma_start(wv_sb[k][:], wv_d[k * 128:(k + 1) * 128, :])

                # A: qkT per m-pair (q tile mp, k tile 4+mp); wqk streamed.
                for mp in range(4):
                    wpair = wabp.tile([128, KT, 2, 128], F32R, tag="wpair")
                    for k in range(KT):
                        nc.sync.dma_start(
                            wpair[:, k, 0, :],
                            wqk_d[k * 128:(k + 1) * 128, mp * 128:(mp + 1) * 128])
                        nc.sync.dma_start(
                            wpair[:, k, 1, :],
                            wqk_d[k * 128:(k + 1) * 128,
                                  512 + mp * 128:512 + (mp + 1) * 128])
                    for half in range(2):          # 0 -> q tile, 1 -> k tile
                        m = mp if half == 0 else 4 + mp
                        pa = ps.tile([128, 1024], F32, tag="stA", bufs=1,
                                     name=f"paA{mp}{half}")
                        pb = ps.tile([128, 1024], F32, tag="stB", bufs=1,
                                     name=f"paB{mp}{half}")
                        banks = [pa[:, 0:512], pa[:, 512:1024],
                                 pb[:, 0:512], pb[:, 512:1024]]
                        for k in range(KT):
                            for n in range(4):
                                nc.tensor.matmul(
                                    banks[n],
                                    wpair[:, k, half, :],
                                    xt_sb[:, k, n * 512:(n + 1) * 512],
                                    start=(k == 0), stop=(k == KT - 1))
                        for n in range(4):
                            dst = qk_sb[m][:, n * 512:(n + 1) * 512]
                            if half == 0:
                                nc.vector.tensor_scalar_add(dst, banks[n],
                                                            bq_sb[:, mp:mp + 1])
                            else:
                                nc.vector.tensor_copy(dst, banks[n])

                # B: v natural layout, 4 accumulators in the "o" psum tag
                for sig in range(4):
                    pv4 = [ps.tile([128, 512], F32, tag="o", bufs=4,
                                   name=f"pb{sig}{i}") for i in range(4)]
                    for k in range(KT):
                        for s4 in range(4):
                            si = sig * 4 + s4
                            nc.tensor.matmul(
                                pv4[s4],
                                xt_sb[:, k, si * 128:(si + 1) * 128],
                                wv_sb[k][:],
                                start=(k == 0), stop=(k == KT - 1))
                    for s4 in range(4):
                        si = sig * 4 + s4
                        nc.vector.tensor_copy(
                            v_sb[si][:, :, 0:HD],
                            pv4[s4][:].rearrange("p (h d) -> p h d", h=HG))
                        nc.vector.tensor_copy(v_sb[si][:, :, HD:HD + 1],
                                              ones_sb[:])

            # ========= phases C+D+E: attention, normalize, projection =======
            with tc.tile_pool(name="attp", bufs=1) as ap, \
                 tc.tile_pool(name="ptp", bufs=3) as ptp, \
                 tc.tile_pool(name="wyp", bufs=1) as wyp, \
                 tc.tile_pool(name="np_", bufs=1) as np_, \
                 tc.tile_pool(name="bcp", bufs=1) as bcp, \
                 tc.tile_pool(name="yp", bufs=2) as yp:
                at_sb = [ap.tile([128, S], F32R, name=f"at{t}", tag=f"at{t}")
                         for t in range(4)]
                wp_sb = [wyp.tile([128, D], F32R, name=f"wp{t}", tag=f"wp{t}")
                        for t in range(4)]
                for t in range(4):
                    nc.sync.dma_start(wp_sb[t][:], wp_d[t * 128:(t + 1) * 128, :])

                for t in range(4):                 # head pair (2t, 2t+1)
                    qT = [qk_sb[t][0:64, :], qk_sb[t][64:128, :]]
                    kT = [qk_sb[4 + t][0:64, :], qk_sb[4 + t][64:128, :]]
                    for sw in range(2):            # pair of s_i chunks
                        po = [ps.tile([128, 512], F32, tag="o", bufs=4,
                                      name=f"po{t}{sw}{i}") for i in range(4)]
                        # po[0], po[1]: head A ic0/ic1; po[2], po[3]: head B
                        for j in range(ST16):
                            stA = ps.tile([128, 1024], F32, tag="stA", bufs=1,
                                          name=f"stA{t}{sw}{j}")
                            stB = ps.tile([128, 1024], F32, tag="stB", bufs=1,
                                          name=f"stB{t}{sw}{j}")
                            for hx, stx in ((0, stA), (1, stB)):
                                for il in range(2):
                                    ic = sw * 2 + il
                                    nc.tensor.matmul(
                                        stx[:, il * 512:(il + 1) * 512],
                                        kT[hx][:, j * 128:(j + 1) * 128],
                                        qT[hx][:, ic * 512:(ic + 1) * 512],
                                        start=True, stop=True)
                            ptA = ptp.tile([128, 1024], F32R, tag="ptA")
                            nc.scalar.activation(ptA[:], stA[:], AF.Exp,
                                                 bias=0.0, scale=1.0)
                            ptB = ptp.tile([128, 1024], F32R, tag="ptB")
                            nc.scalar.activation(ptB[:], stB[:], AF.Exp,
                                                 bias=0.0, scale=1.0)
                            for hx, ptx in ((0, ptA), (1, ptB)):
                                for il in range(2):
                                    nc.tensor.matmul(
                                        po[2 * hx + il][0:HD + 1, :],
                                        v_sb[j][:, 2 * t + hx, :],
                                        ptx[:, il * 512:(il + 1) * 512],
                                        start=(j == 0), stop=(j == ST16 - 1))
                        # normalize: inv = exp(-ln Z); broadcast; multiply
                        # evacuate po fast (psum banks are the scarce resource):
                        # unnormalized attn rows + Z row, then normalize in
                        # SBUF off the critical path.
                        za = np_.tile([1, 2048], F32, tag="za")
                        slots = []
                        for hx in range(2):
                            for il in range(2):
                                i = 2 * hx + il
                                ic = sw * 2 + il
                                sl = at_sb[t][hx * 64:hx * 64 + 64,
                                              ic * 512:(ic + 1) * 512]
                                slots.append(sl)
                                nc.vector.tensor_copy(sl, po[i][0:64, :])
                                nc.vector.tensor_copy(
                                    za[:, i * 512:(i + 1) * 512],
                                    po[i][64:65, :])
                        zb = np_.tile([1, 2048], F32, tag="zb")
                        nc.scalar.activation(zb[:], za[:], AF.Ln,
                                             bias=0.0, scale=1.0)
                        inv = np_.tile([1, 2048], F32, tag="za")
                        nc.scalar.activation(inv[:], zb[:], AF.Exp,
                                             bias=0.0, scale=-1.0)
                        bc = bcp.tile([128, 2048], F32, tag="bc")
                        nc.gpsimd.partition_broadcast(bc[:], inv[:])
                        for hx in range(2):
                            for il in range(2):
                                i = 2 * hx + il
                                nc.vector.tensor_mul(
                                    slots[i], slots[i],
                                    bc[hx * 64:hx * 64 + 64,
                                       i * 512:(i + 1) * 512])

                # E: out = attnT.T @ wp
                for si in range(ST16):
                    py = [ps.tile([128, 512], F32, tag="o", bufs=4,
                                  name=f"py{si}{i}") for i in range(2)]
                    for t in range(4):
                        for nch in range(2):
                            nc.tensor.matmul(
                                py[nch],
                                at_sb[t][:, si * 128:(si + 1) * 128],
                                wp_sb[t][:, nch * 512:(nch + 1) * 512],
                                start=(t == 0), stop=(t == 3))
                    for nch in range(2):
                        y = yp.tile([128, 512], F32, tag="y")
                        nc.vector.tensor_copy(y[:], py[nch][:])
                        nc.sync.dma_start(
                            out_d[si * 128:(si + 1) * 128,
                                  nch * 512:(nch + 1) * 512], y[:])
    nc.compile()
    return nc


def _prep_inputs(x, w_qkv, b_qkv, w_proj):
    """Host-side shard prep: slice per head-group, fold scale, transpose x."""
    in_maps = []
    for c in range(N_CORES):
        b, g = c // 2, c % 2
        cs = g * 512
        wq = w_qkv[:, cs:cs + 512] * 0.125
        wk = w_qkv[:, 1024 + cs:1024 + cs + 512]
        wv = w_qkv[:, 2048 + cs:2048 + cs + 512]
        bq = (b_qkv[cs:cs + 512] * 0.125).reshape(4, 128).T
        in_maps.append({
            "xt": np.ascontiguousarray(x[b].T),
            "wqk": np.ascontiguousarray(np.concatenate([wq, wk], axis=1)),
            "wv": np.ascontiguousarray(wv),
            "wp": np.ascontiguousarray(w_proj[g * 512:(g + 1) * 512, :]),
            "bq": np.ascontiguousarray(bq.astype(np.float32)),
        })
    return in_maps


def kernel(x, w_qkv, b_qkv, w_proj, b_proj, _trace=False):
    x = np.asarray(x, np.float32)
    w_qkv = np.asarray(w_qkv, np.float32)
    b_qkv = np.asarray(b_qkv, np.float32)
    w_proj = np.asarray(w_proj, np.float32)
    b_proj = np.asarray(b_proj, np.float32)

    if "nc" not in _CACHE:
        _CACHE["nc"] = _build()
    nc = _CACHE["nc"]

    in_maps = _prep_inputs(x, w_qkv, b_qkv, w_proj)
    res = bass_utils.run_bass_kernel_spmd(
        nc, in_maps, core_ids=list(range(N_CORES)), trace=_trace)

    # host-side bias: b_proj plus the value-bias path through w_proj
    bias = b_proj + b_qkv[2048:3072].astype(np.float64) @ w_proj.astype(np.float64)
    bias = bias.astype(np.float32)
    out = np.empty((B, S, D), np.float32)
    for b in range(B):
        out[b] = res.results[2 * b]["out"] + res.results[2 * b + 1]["out"] + bias
    if _trace:
        return out, res
    return out


# revision 16
# speedup vs baseline: 1.0227x; 1.0227x over previous
"""Multi-head attention forward on 8 Trainium2 NeuronCores.

Sharding: core c = 2*b + g handles batch b (of 4) and head-group g (8 of 16
heads). Each core computes its group's attention output projected through its
slice of w_proj (row-parallel); the host sums the two partial products per
batch and adds the bias terms.

Math notes (exact identities, not approximations):
  - the key bias b_k adds a per-query constant to every score row, which
    softmax ignores;
  - the value bias b_v passes through attention unchanged (attn rows sum to 1)
    so its projection b_v @ w_proj is folded into the host-side bias;
  - the 1/sqrt(64) score scale is folded into w_q / b_q (exact: power of two).

Compute dtype is float32r (TensorE reduced-precision fp32 path): ~bf16 matmul
speed at 512-wide moving operands, ~2e-4 error instead of bf16's ~4e-3.

Layouts: scores are computed transposed (ST = kT.T @ qT, [s_j, s_i]) so the
exp output PT feeds the PV matmul directly as the moving operand with no
P-transpose pass. V carries an appended ones column, so the PV output row 64
is the softmax denominator Z. Normalization (1/Z) happens on the PV psum via
ACT ln->exp + GPSIMD partition-broadcast. Head pairs (2t, 2t+1) live in the
partition halves {0..63} / {64..127} of shared qk tiles, so their K=64 score
matmuls land in different PE row groups and execute concurrently.
"""

import numpy as np

import concourse.bass as bass
import concourse.tile as tile
from concourse import bacc, mybir
from concourse import bass_utils

F32 = mybir.dt.float32
F32R = mybir.dt.float32r
AF = mybir.ActivationFunctionType

B, S, D = 4, 2048, 1024
H, HD = 16, 64
HG = 8            # heads per core (group)
N_CORES = 8

_CACHE = {}


def _build():
    nc = bacc.Bacc("TRN2", target_bir_lowering=False, debug=False,
                   num_devices=N_CORES)
    xt_d = nc.dram_tensor("xt", [D, S], F32R, kind="ExternalInput").ap()
    wqk_d = nc.dram_tensor("wqk", [D, 2 * HG * HD], F32R, kind="ExternalInput").ap()
    wv_d = nc.dram_tensor("wv", [D, HG * HD], F32R, kind="ExternalInput").ap()
    wp_d = nc.dram_tensor("wp", [HG * HD, D], F32R, kind="ExternalInput").ap()
    bq_d = nc.dram_tensor("bq", [128, 4], F32, kind="ExternalInput").ap()
    out_d = nc.dram_tensor("out", [S, D], F32, kind="ExternalOutput").ap()

    KT = D // 128          # 8 k-tiles over the embedding dim
    ST16 = S // 128        # 16 tiles over sequence

    with tile.TileContext(nc) as tc:
        with (
            tc.tile_pool(name="persist", bufs=1) as pp,
            tc.tile_pool(name="psum", bufs=1, space="PSUM") as ps,
        ):
            # ---- persistent SBUF tensors ----
            qk_sb = [pp.tile([128, S], F32R, name=f"qk{m}", tag=f"qk{m}")
                     for m in range(8)]
            v_sb = [pp.tile([128, HG, HD + 1], F32R, name=f"v{j}", tag=f"v{j}")
                    for j in range(ST16)]
            bq_sb = pp.tile([128, 4], F32, tag="bq")
            nc.sync.dma_start(bq_sb[:], bq_d)
            ones_sb = pp.tile([128, HG, 1], F32, tag="ones")
            nc.vector.memset(ones_sb[:], 1.0)

            # ============ phases A+B: qkT and v (xt fully resident) =========
            with tc.tile_pool(name="xtp", bufs=1) as xtp, \
                 tc.tile_pool(name="wabp", bufs=1) as wabp:
                xt_sb = xtp.tile([128, KT, S], F32R, tag="xt")
                for k in range(KT):
                    nc.sync.dma_start(xt_sb[:, k, :], xt_d[k * 128:(k + 1) * 128, :])
                wv_sb = [wabp.tile([128, 512], F32R, name=f"wv{k}", tag=f"wv{k}")
                         for k in range(KT)]
                for k in range(KT):
                    nc.sync.dma_start(wv_sb[k][:], wv_d[k * 128:(k + 1) * 128, :])

                # A: qkT per m-pair (q tile mp, k tile 4+mp); wqk streamed.
                for mp in range(4):
                    wpair = wabp.tile([128, KT, 2, 128], F32R, tag="wpair")
                    for k in range(KT):
                        nc.sync.dma_start(
                            wpair[:, k, 0, :],
                            wqk_d[k * 128:(k + 1) * 128, mp * 128:(mp + 1) * 128])
                        nc.sync.dma_start(
                            wpair[:, k, 1, :],
                            wqk_d[k * 128:(k + 1) * 128,
                                  512 + mp * 128:512 + (mp + 1) * 128])
                    for half in range(2):          # 0 -> q tile, 1 -> k tile
                        m = mp if half == 0 else 4 + mp
                        pa = ps.tile([128, 1024], F32, tag="st", bufs=2,
                                     name=f"paA{mp}{half}")
                        pb = ps.tile([128, 1024], F32, tag="st", bufs=2,
                                     name=f"paB{mp}{half}")
                        banks = [pa[:, 0:512], pa[:, 512:1024],
                                 pb[:, 0:512], pb[:, 512:1024]]
                        for k in range(KT):
                            for n in range(4):
                                nc.tensor.matmul(
                                    banks[n],
                                    wpair[:, k, half, :],
                                    xt_sb[:, k, n * 512:(n + 1) * 512],
                                    start=(k == 0), stop=(k == KT - 1))
                        for n in range(4):
                            dst = qk_sb[m][:, n * 512:(n + 1) * 512]
                            if half == 0:
                                nc.vector.tensor_scalar_add(dst, banks[n],
                                                            bq_sb[:, mp:mp + 1])
                            else:
                                nc.vector.tensor_copy(dst, banks[n])

                # B: v natural layout, 4 accumulators in the "o" psum tag
                for sig in range(4):
                    pv4 = [ps.tile([128, 512], F32, tag="o", bufs=4,
                                   name=f"pb{sig}{i}") for i in range(4)]
                    for k in range(KT):
                        for s4 in range(4):
                            si = sig * 4 + s4
                            nc.tensor.matmul(
                                pv4[s4],
                                xt_sb[:, k, si * 128:(si + 1) * 128],
                                wv_sb[k][:],
                                start=(k == 0), stop=(k == KT - 1))
                    for s4 in range(4):
                        si = sig * 4 + s4
                        nc.vector.tensor_copy(
                            v_sb[si][:, :, 0:HD],
                            pv4[s4][:].rearrange("p (h d) -> p h d", h=HG))
                        nc.vector.tensor_copy(v_sb[si][:, :, HD:HD + 1],
                                              ones_sb[:])

            # ========= phases C+D+E: attention, normalize, projection =======
            with tc.tile_pool(name="attp", bufs=1) as ap, \
                 tc.tile_pool(name="ptp", bufs=3) as ptp, \
                 tc.tile_pool(name="wyp", bufs=1) as wyp, \
                 tc.tile_pool(name="np_", bufs=1) as np_, \
                 tc.tile_pool(name="bcp", bufs=1) as bcp, \
                 tc.tile_pool(name="yp", bufs=2) as yp:
                at_sb = [ap.tile([128, S], F32R, name=f"at{t}", tag=f"at{t}")
                         for t in range(4)]
                wp_sb = [wyp.tile([128, D], F32R, name=f"wp{t}", tag=f"wp{t}")
                        for t in range(4)]
                for t in range(4):
                    nc.sync.dma_start(wp_sb[t][:], wp_d[t * 128:(t + 1) * 128, :])

                for t in range(4):                 # head pair (2t, 2t+1)
                    qT = [qk_sb[t][0:64, :], qk_sb[t][64:128, :]]
                    kT = [qk_sb[4 + t][0:64, :], qk_sb[4 + t][64:128, :]]
                    for sw in range(2):            # pair of s_i chunks
                        po = [ps.tile([128, 512], F32, tag="o", bufs=4,
                                      name=f"po{t}{sw}{i}") for i in range(4)]
                        # po[0], po[1]: head A ic0/ic1; po[2], po[3]: head B
                        # Flat software-pipelined stream over (j, head): STs
                        # run one step ahead of PVs so the PE never waits on
                        # the exp in steady state; alternating heads put
                        # consecutive K=64 matmuls in different PE row groups.
                        pts = {}
                        for j in range(ST16 + 1):
                            if j < ST16:
                                for hx in range(2):
                                    st = ps.tile([128, 1024], F32, tag="st",
                                                 bufs=2, name=f"st{t}{sw}{j}{hx}")
                                    for il in range(2):
                                        ic = sw * 2 + il
                                        nc.tensor.matmul(
                                            st[:, il * 512:(il + 1) * 512],
                                            kT[hx][:, j * 128:(j + 1) * 128],
                                            qT[hx][:, ic * 512:(ic + 1) * 512],
                                            start=True, stop=True)
                                    pt = ptp.tile([128, 1024], F32R, tag="pt",
                                                  bufs=4,
                                                  name=f"pt{t}{sw}{j}{hx}")
                                    nc.scalar.activation(pt[:], st[:], AF.Exp,
                                                         bias=0.0, scale=1.0)
                                    pts[hx] = pt
                            if j > 0:
                                for hx in range(2):
                                    for il in range(2):
                                        nc.tensor.matmul(
                                            po[2 * hx + il][0:HD + 1, :],
                                            v_sb[j - 1][:, 2 * t + hx, :],
                                            pts_prev[hx][:, il * 512:(il + 1) * 512],
                                            start=(j == 1), stop=(j == ST16))
                            pts_prev = dict(pts)
                        # normalize: inv = exp(-ln Z); broadcast; multiply
                        # evacuate po fast (psum banks are the scarce resource):
                        # unnormalized attn rows + Z row, then normalize in
                        # SBUF off the critical path.
                        za = np_.tile([1, 2048], F32, tag="za")
                        slots = []
                        for hx in range(2):
                            for il in range(2):
                                i = 2 * hx + il
                                ic = sw * 2 + il
                                sl = at_sb[t][hx * 64:hx * 64 + 64,
                                              ic * 512:(ic + 1) * 512]
                                slots.append(sl)
                                nc.vector.tensor_copy(sl, po[i][0:64, :])
                                nc.vector.tensor_copy(
                                    za[:, i * 512:(i + 1) * 512],
                                    po[i][64:65, :])
                        inv = np_.tile([1, 2048], F32, tag="zb")
                        nc.vector.reciprocal(inv[:], za[:])
                        bc = bcp.tile([128, 2048], F32, tag="bc")
                        nc.gpsimd.partition_broadcast(bc[:], inv[:])
                        for hx in range(2):
                            for il in range(2):
                                i = 2 * hx + il
                                nc.vector.tensor_mul(
                                    slots[i], slots[i],
                                    bc[hx * 64:hx * 64 + 64,
                                       i * 512:(i + 1) * 512])

                # E: out = attnT.T @ wp
                for si in range(ST16):
                    py = [ps.tile([128, 512], F32, tag="o", bufs=4,
                                  name=f"py{si}{i}") for i in range(2)]
                    for t in range(4):
                        for nch in range(2):
                            nc.tensor.matmul(
                                py[nch],
                                at_sb[t][:, si * 128:(si + 1) * 128],
                                wp_sb[t][:, nch * 512:(nch + 1) * 512],
                                start=(t == 0), stop=(t == 3))
                    for nch in range(2):
                        y = yp.tile([128, 512], F32, tag="y")
                        nc.vector.tensor_copy(y[:], py[nch][:])
                        nc.sync.dma_start(
                            out_d[si * 128:(si + 1) * 128,
                                  nch * 512:(nch + 1) * 512], y[:])
    nc.compile()
    return nc


def _prep_inputs(x, w_qkv, b_qkv, w_proj):
    """Host-side shard prep: slice per head-group, fold scale, transpose x."""
    in_maps = []
    for c in range(N_CORES):
        b, g = c // 2, c % 2
        cs = g * 512
        wq = w_qkv[:, cs:cs + 512] * 0.125
        wk = w_qkv[:, 1024 + cs:1024 + cs + 512]
        wv = w_qkv[:, 2048 + cs:2048 + cs + 512]
        bq = (b_qkv[cs:cs + 512] * 0.125).reshape(4, 128).T
        in_maps.append({
            "xt": np.ascontiguousarray(x[b].T),
            "wqk": np.ascontiguousarray(np.concatenate([wq, wk], axis=1)),
            "wv": np.ascontiguousarray(wv),
            "wp": np.ascontiguousarray(w_proj[g * 512:(g + 1) * 512, :]),
            "bq": np.ascontiguousarray(bq.astype(np.float32)),
        })
    return in_maps


def kernel(x, w_qkv, b_qkv, w_proj, b_proj, _trace=False):
    x = np.asarray(x, np.float32)
    w_qkv = np.asarray(w_qkv, np.float32)
    b_qkv = np.asarray(b_qkv, np.float32)
    w_proj = np.asarray(w_proj, np.float32)
    b_proj = np.asarray(b_proj, np.float32)

    if "nc" not in _CACHE:
        _CACHE["nc"] = _build()
    nc = _CACHE["nc"]

    in_maps = _prep_inputs(x, w_qkv, b_qkv, w_proj)
    res = bass_utils.run_bass_kernel_spmd(
        nc, in_maps, core_ids=list(range(N_CORES)), trace=_trace)

    # host-side bias: b_proj plus the value-bias path through w_proj
    bias = b_proj + b_qkv[2048:3072].astype(np.float64) @ w_proj.astype(np.float64)
    bias = bias.astype(np.float32)
    out = np.empty((B, S, D), np.float32)
    for b in range(B):
        out[b] = res.results[2 * b]["out"] + res.results[2 * b + 1]["out"] + bias
    if _trace:
        return out, res
    return out


# revision 17
# speedup vs baseline: 1.4685x; 1.4359x over previous
"""Multi-head attention forward on 8 Trainium2 NeuronCores.

Sharding: core c = 2*b + g handles batch b (of 4) and head-group g (8 of 16
heads). Each core computes its group's attention output projected through its
slice of w_proj (row-parallel); the host sums the two partial products per
batch and adds the bias terms.

Math notes (exact identities, not approximations):
  - the key bias b_k adds a per-query constant to every score row, which
    softmax ignores;
  - the value bias b_v passes through attention unchanged (attn rows sum to 1)
    so its projection b_v @ w_proj is folded into the host-side bias;
  - the 1/sqrt(64) score scale is folded into w_q / b_q (exact: power of two).

Compute dtype is float32r (TensorE reduced-precision fp32 path): ~bf16 matmul
speed at 512-wide moving operands, ~2e-4 error instead of bf16's ~4e-3.

Layouts: scores are computed transposed (ST = kT.T @ qT, [s_j, s_i]) so the
exp output PT feeds the PV matmul directly as the moving operand with no
P-transpose pass. V carries an appended ones column, so the PV output row 64
is the softmax denominator Z. Normalization (1/Z) happens on the PV psum via
ACT ln->exp + GPSIMD partition-broadcast. Head pairs (2t, 2t+1) live in the
partition halves {0..63} / {64..127} of shared qk tiles, so their K=64 score
matmuls land in different PE row groups and execute concurrently.
"""

import numpy as np

import concourse.bass as bass
import concourse.tile as tile
from concourse import bacc, mybir
from concourse import bass_utils

F32 = mybir.dt.float32
F32R = mybir.dt.float32r
AF = mybir.ActivationFunctionType

B, S, D = 4, 2048, 1024
H, HD = 16, 64
HG = 8            # heads per core (group)
N_CORES = 8

_CACHE = {}


def _build():
    nc = bacc.Bacc("TRN2", target_bir_lowering=False, debug=False,
                   num_devices=N_CORES)
    xt_d = nc.dram_tensor("xt", [D, S], F32R, kind="ExternalInput").ap()
    wqk_d = nc.dram_tensor("wqk", [D, 2 * HG * HD], F32R, kind="ExternalInput").ap()
    wv_d = nc.dram_tensor("wv", [D, HG * HD], F32R, kind="ExternalInput").ap()
    wp_d = nc.dram_tensor("wp", [HG * HD, D], F32R, kind="ExternalInput").ap()
    bq_d = nc.dram_tensor("bq", [128, 4], F32, kind="ExternalInput").ap()
    out_d = nc.dram_tensor("out", [S, D], F32, kind="ExternalOutput").ap()

    KT = D // 128          # 8 k-tiles over the embedding dim
    ST16 = S // 128        # 16 tiles over sequence

    with tile.TileContext(nc) as tc:
        with (
            tc.tile_pool(name="persist", bufs=1) as pp,
            tc.tile_pool(name="psum", bufs=1, space="PSUM") as ps,
        ):
            # ---- persistent SBUF tensors ----
            qk_sb = [pp.tile([128, S], F32R, name=f"qk{m}", tag=f"qk{m}")
                     for m in range(8)]
            v_sb = [pp.tile([128, HG, HD + 1], F32R, name=f"v{j}", tag=f"v{j}")
                    for j in range(ST16)]
            bq_sb = pp.tile([128, 4], F32, tag="bq")
            nc.sync.dma_start(bq_sb[:], bq_d)
            ones_sb = pp.tile([128, HG, 1], F32, tag="ones")
            nc.vector.memset(ones_sb[:], 1.0)

            # ============ phases A+B: qkT and v (xt fully resident) =========
            with tc.tile_pool(name="xtp", bufs=1) as xtp, \
                 tc.tile_pool(name="wabp", bufs=1) as wabp:
                xt_sb = xtp.tile([128, KT, S], F32R, tag="xt")
                for k in range(KT):
                    nc.sync.dma_start(xt_sb[:, k, :], xt_d[k * 128:(k + 1) * 128, :])
                wv_sb = [wabp.tile([128, 512], F32R, name=f"wv{k}", tag=f"wv{k}")
                         for k in range(KT)]
                for k in range(KT):
                    nc.sync.dma_start(wv_sb[k][:], wv_d[k * 128:(k + 1) * 128, :])

                # A: qkT per m-pair (q tile mp, k tile 4+mp); wqk streamed.
                for mp in range(4):
                    wpair = wabp.tile([128, KT, 2, 128], F32R, tag="wpair")
                    for k in range(KT):
                        nc.sync.dma_start(
                            wpair[:, k, 0, :],
                            wqk_d[k * 128:(k + 1) * 128, mp * 128:(mp + 1) * 128])
                        nc.sync.dma_start(
                            wpair[:, k, 1, :],
                            wqk_d[k * 128:(k + 1) * 128,
                                  512 + mp * 128:512 + (mp + 1) * 128])
                    for half in range(2):          # 0 -> q tile, 1 -> k tile
                        m = mp if half == 0 else 4 + mp
                        pa = ps.tile([128, 1024], F32, tag="st", bufs=2,
                                     name=f"paA{mp}{half}")
                        pb = ps.tile([128, 1024], F32, tag="st", bufs=2,
                                     name=f"paB{mp}{half}")
                        banks = [pa[:, 0:512], pa[:, 512:1024],
                                 pb[:, 0:512], pb[:, 512:1024]]
                        for k in range(KT):
                            for n in range(4):
                                nc.tensor.matmul(
                                    banks[n],
                                    wpair[:, k, half, :],
                                    xt_sb[:, k, n * 512:(n + 1) * 512],
                                    start=(k == 0), stop=(k == KT - 1))
                        for n in range(4):
                            dst = qk_sb[m][:, n * 512:(n + 1) * 512]
                            if half == 0:
                                nc.vector.tensor_scalar_add(dst, banks[n],
                                                            bq_sb[:, mp:mp + 1])
                            else:
                                nc.vector.tensor_copy(dst, banks[n])

                # B: v natural layout, 4 accumulators in the "o" psum tag
                for sig in range(4):
                    pv4 = [ps.tile([128, 512], F32, tag="o", bufs=4,
                                   name=f"pb{sig}{i}") for i in range(4)]
                    for k in range(KT):
                        for s4 in range(4):
                            si = sig * 4 + s4
                            nc.tensor.matmul(
                                pv4[s4],
                                xt_sb[:, k, si * 128:(si + 1) * 128],
                                wv_sb[k][:],
                                start=(k == 0), stop=(k == KT - 1))
                    for s4 in range(4):
                        si = sig * 4 + s4
                        nc.vector.tensor_copy(
                            v_sb[si][:, :, 0:HD],
                            pv4[s4][:].rearrange("p (h d) -> p h d", h=HG))
                        nc.vector.tensor_copy(v_sb[si][:, :, HD:HD + 1],
                                              ones_sb[:])

            # ========= phases C+D+E: attention, normalize, projection =======
            with tc.tile_pool(name="attp", bufs=1) as ap, \
                 tc.tile_pool(name="ptp", bufs=3) as ptp, \
                 tc.tile_pool(name="wyp", bufs=1) as wyp, \
                 tc.tile_pool(name="np_", bufs=1) as np_, \
                 tc.tile_pool(name="bcp", bufs=1) as bcp, \
                 tc.tile_pool(name="yp", bufs=2) as yp:
                at_sb = [ap.tile([128, S], F32R, name=f"at{t}", tag=f"at{t}")
                         for t in range(4)]
                wp_sb = [wyp.tile([128, D], F32R, name=f"wp{t}", tag=f"wp{t}")
                        for t in range(4)]
                for t in range(4):
                    nc.sync.dma_start(wp_sb[t][:], wp_d[t * 128:(t + 1) * 128, :])

                for h in range(HG):
                    mt = h // 2
                    r0 = (h % 2) * 64
                    qT = qk_sb[mt][r0:r0 + 64, :]
                    kT = qk_sb[4 + mt][r0:r0 + 64, :]
                    for sw in range(2):            # pair of s_i chunks
                        po = [ps.tile([128, 512], F32, tag="o", bufs=4,
                                      name=f"po{h}{sw}{i}") for i in range(2)]
                        for j in range(ST16):
                            st = ps.tile([128, 1024], F32, tag="st",
                                         bufs=2, name=f"st{h}{sw}{j}")
                            for il in range(2):
                                ic = sw * 2 + il
                                nc.tensor.matmul(
                                    st[:, il * 512:(il + 1) * 512],
                                    kT[:, j * 128:(j + 1) * 128],
                                    qT[:, ic * 512:(ic + 1) * 512],
                                    start=True, stop=True)
                            pt = ptp.tile([128, 1024], F32R, tag="pt",
                                          bufs=3, name=f"pt{h}{sw}{j}")
                            nc.scalar.activation(pt[:], st[:], AF.Exp,
                                                 bias=0.0, scale=1.0)
                            for il in range(2):
                                nc.tensor.matmul(
                                    po[il][0:HD + 1, :],
                                    v_sb[j][:, h, :],
                                    pt[:, il * 512:(il + 1) * 512],
                                    start=(j == 0), stop=(j == ST16 - 1))
                        # evacuate po fast (psum banks are the scarce
                        # resource): unnormalized attn rows + Z row, then
                        # normalize in SBUF off the critical path.
                        za = np_.tile([1, 1024], F32, tag="za")
                        slots = []
                        for il in range(2):
                            ic = sw * 2 + il
                            sl = at_sb[mt][r0:r0 + 64,
                                           ic * 512:(ic + 1) * 512]
                            slots.append(sl)
                            nc.vector.tensor_copy(sl, po[il][0:64, :])
                            nc.vector.tensor_copy(
                                za[:, il * 512:(il + 1) * 512],
                                po[il][64:65, :])
                        inv = np_.tile([1, 1024], F32, tag="zb")
                        nc.vector.reciprocal(inv[:], za[:])
                        bc = bcp.tile([128, 1024], F32, tag="bc")
                        nc.gpsimd.partition_broadcast(bc[:], inv[:])
                        for il in range(2):
                            nc.vector.tensor_mul(
                                slots[il], slots[il],
                                bc[r0:r0 + 64, il * 512:(il + 1) * 512])

                # E: out = attnT.T @ wp
                for si in range(ST16):
                    py = [ps.tile([128, 512], F32, tag="o", bufs=4,
                                  name=f"py{si}{i}") for i in range(2)]
                    for t in range(4):
                        for nch in range(2):
                            nc.tensor.matmul(
                                py[nch],
                                at_sb[t][:, si * 128:(si + 1) * 128],
                                wp_sb[t][:, nch * 512:(nch + 1) * 512],
                                start=(t == 0), stop=(t == 3))
                    for nch in range(2):
                        y = yp.tile([128, 512], F32, tag="y")
                        nc.vector.tensor_copy(y[:], py[nch][:])
                        nc.sync.dma_start(
                            out_d[si * 128:(si + 1) * 128,
                                  nch * 512:(nch + 1) * 512], y[:])
    nc.compile()
    return nc


def _prep_inputs(x, w_qkv, b_qkv, w_proj):
    """Host-side shard prep: slice per head-group, fold scale, transpose x."""
    in_maps = []
    for c in range(N_CORES):
        b, g = c // 2, c % 2
        cs = g * 512
        wq = w_qkv[:, cs:cs + 512] * 0.125
        wk = w_qkv[:, 1024 + cs:1024 + cs + 512]
        wv = w_qkv[:, 2048 + cs:2048 + cs + 512]
        bq = (b_qkv[cs:cs + 512] * 0.125).reshape(4, 128).T
        in_maps.append({
            "xt": np.ascontiguousarray(x[b].T),
            "wqk": np.ascontiguousarray(np.concatenate([wq, wk], axis=1)),
            "wv": np.ascontiguousarray(wv),
            "wp": np.ascontiguousarray(w_proj[g * 512:(g + 1) * 512, :]),
            "bq": np.ascontiguousarray(bq.astype(np.float32)),
        })
    return in_maps


def kernel(x, w_qkv, b_qkv, w_proj, b_proj, _trace=False):
    x = np.asarray(x, np.float32)
    w_qkv = np.asarray(w_qkv, np.float32)
    b_qkv = np.asarray(b_qkv, np.float32)
    w_proj = np.asarray(w_proj, np.float32)
    b_proj = np.asarray(b_proj, np.float32)

    if "nc" not in _CACHE:
        _CACHE["nc"] = _build()
    nc = _CACHE["nc"]

    in_maps = _prep_inputs(x, w_qkv, b_qkv, w_proj)
    res = bass_utils.run_bass_kernel_spmd(
        nc, in_maps, core_ids=list(range(N_CORES)), trace=_trace)

    # host-side bias: b_proj plus the value-bias path through w_proj
    bias = b_proj + b_qkv[2048:3072].astype(np.float64) @ w_proj.astype(np.float64)
    bias = bias.astype(np.float32)
    out = np.empty((B, S, D), np.float32)
    for b in range(B):
        out[b] = res.results[2 * b]["out"] + res.results[2 * b + 1]["out"] + bias
    if _trace:
        return out, res
    return out


# revision 18
# speedup vs baseline: 1.4854x; 1.0115x over previous
"""Multi-head attention forward on 8 Trainium2 NeuronCores.

Sharding: core c = 2*b + g handles batch b (of 4) and head-group g (8 of 16
heads). Each core computes its group's attention output projected through its
slice of w_proj (row-parallel); the host sums the two partial products per
batch and adds the bias terms.

Math notes (exact identities, not approximations):
  - the key bias b_k adds a per-query constant to every score row, which
    softmax ignores;
  - the value bias b_v passes through attention unchanged (attn rows sum to 1)
    so its projection b_v @ w_proj is folded into the host-side bias;
  - the 1/sqrt(64) score scale is folded into w_q / b_q (exact: power of two).

Compute dtype is float32r (TensorE reduced-precision fp32 path): ~bf16 matmul
speed at 512-wide moving operands, ~2e-4 error instead of bf16's ~4e-3.

Layouts: scores are computed transposed (ST = kT.T @ qT, [s_j, s_i]) so the
exp output PT feeds the PV matmul directly as the moving operand with no
P-transpose pass. V carries an appended ones column, so the PV output row 64
is the softmax denominator Z. Normalization (1/Z) happens on the PV psum via
ACT ln->exp + GPSIMD partition-broadcast. Head pairs (2t, 2t+1) live in the
partition halves {0..63} / {64..127} of shared qk tiles, so their K=64 score
matmuls land in different PE row groups and execute concurrently.
"""

import numpy as np

import concourse.bass as bass
import concourse.tile as tile
from concourse import bacc, mybir
from concourse import bass_utils

F32 = mybir.dt.float32
F32R = mybir.dt.float32r
AF = mybir.ActivationFunctionType

B, S, D = 4, 2048, 1024
H, HD = 16, 64
HG = 8            # heads per core (group)
N_CORES = 8

_CACHE = {}


def _build():
    nc = bacc.Bacc("TRN2", target_bir_lowering=False, debug=False,
                   num_devices=N_CORES)
    xt_d = nc.dram_tensor("xt", [D, S], F32R, kind="ExternalInput").ap()
    wqk_d = nc.dram_tensor("wqk", [D, 2 * HG * HD], F32R, kind="ExternalInput").ap()
    wv_d = nc.dram_tensor("wv", [D, HG * HD], F32R, kind="ExternalInput").ap()
    wp_d = nc.dram_tensor("wp", [HG * HD, D], F32R, kind="ExternalInput").ap()
    bq_d = nc.dram_tensor("bq", [128, 4], F32, kind="ExternalInput").ap()
    out_d = nc.dram_tensor("out", [S, D], F32, kind="ExternalOutput").ap()

    KT = D // 128          # 8 k-tiles over the embedding dim
    ST16 = S // 128        # 16 tiles over sequence

    with tile.TileContext(nc) as tc:
        with (
            tc.tile_pool(name="persist", bufs=1) as pp,
            tc.tile_pool(name="psum", bufs=1, space="PSUM") as ps,
        ):
            # ---- persistent SBUF tensors ----
            qk_sb = [pp.tile([128, S], F32R, name=f"qk{m}", tag=f"qk{m}")
                     for m in range(8)]
            v_sb = [pp.tile([128, HG, HD + 1], F32R, name=f"v{j}", tag=f"v{j}")
                    for j in range(ST16)]
            bq_sb = pp.tile([128, 4], F32, tag="bq")
            nc.sync.dma_start(bq_sb[:], bq_d)
            ones_sb = pp.tile([128, HG, 1], F32, tag="ones")
            nc.vector.memset(ones_sb[:], 1.0)

            # ============ phases A+B: qkT and v (xt fully resident) =========
            with tc.tile_pool(name="xtp", bufs=1) as xtp, \
                 tc.tile_pool(name="wabp", bufs=1) as wabp:
                xt_sb = [xtp.tile([128, S], F32R, name=f"xt{k}", tag=f"xt{k}")
                         for k in range(KT)]
                for k in range(KT):
                    nc.sync.dma_start(xt_sb[k][:], xt_d[k * 128:(k + 1) * 128, :])
                wv_sb = [wabp.tile([128, 512], F32R, name=f"wv{k}", tag=f"wv{k}")
                         for k in range(KT)]
                for k in range(KT):
                    nc.sync.dma_start(wv_sb[k][:], wv_d[k * 128:(k + 1) * 128, :])

                # A: qkT per m-pair (q tile mp, k tile 4+mp); wqk streamed.
                for mp in range(4):
                    wpair = wabp.tile([128, KT, 2, 128], F32R, tag="wpair", bufs=2)
                    for k in range(KT):
                        nc.sync.dma_start(
                            wpair[:, k, 0, :],
                            wqk_d[k * 128:(k + 1) * 128, mp * 128:(mp + 1) * 128])
                        nc.sync.dma_start(
                            wpair[:, k, 1, :],
                            wqk_d[k * 128:(k + 1) * 128,
                                  512 + mp * 128:512 + (mp + 1) * 128])
                    for half in range(2):          # 0 -> q tile, 1 -> k tile
                        m = mp if half == 0 else 4 + mp
                        pa = ps.tile([128, 1024], F32, tag="st", bufs=2,
                                     name=f"paA{mp}{half}")
                        pb = ps.tile([128, 1024], F32, tag="st", bufs=2,
                                     name=f"paB{mp}{half}")
                        banks = [pa[:, 0:512], pa[:, 512:1024],
                                 pb[:, 0:512], pb[:, 512:1024]]
                        for k in range(KT):
                            for n in range(4):
                                nc.tensor.matmul(
                                    banks[n],
                                    wpair[:, k, half, :],
                                    xt_sb[k][:, n * 512:(n + 1) * 512],
                                    start=(k == 0), stop=(k == KT - 1))
                        for n in range(4):
                            dst = qk_sb[m][:, n * 512:(n + 1) * 512]
                            if half == 0:
                                nc.vector.tensor_scalar_add(dst, banks[n],
                                                            bq_sb[:, mp:mp + 1])
                            else:
                                nc.vector.tensor_copy(dst, banks[n])

                # B: v natural layout, 4 accumulators in the "o" psum tag
                for sig in range(4):
                    pv4 = [ps.tile([128, 512], F32, tag="o", bufs=4,
                                   name=f"pb{sig}{i}") for i in range(4)]
                    for k in range(KT):
                        for s4 in range(4):
                            si = sig * 4 + s4
                            nc.tensor.matmul(
                                pv4[s4],
                                xt_sb[k][:, si * 128:(si + 1) * 128],
                                wv_sb[k][:],
                                start=(k == 0), stop=(k == KT - 1))
                    for s4 in range(4):
                        si = sig * 4 + s4
                        nc.vector.tensor_copy(
                            v_sb[si][:, :, 0:HD],
                            pv4[s4][:].rearrange("p (h d) -> p h d", h=HG))
                        nc.vector.tensor_copy(v_sb[si][:, :, HD:HD + 1],
                                              ones_sb[:])

            # ========= phases C+D+E: attention, normalize, projection =======
            with tc.tile_pool(name="attp", bufs=1) as ap, \
                 tc.tile_pool(name="ptp", bufs=3) as ptp, \
                 tc.tile_pool(name="wyp", bufs=1) as wyp, \
                 tc.tile_pool(name="np_", bufs=1) as np_, \
                 tc.tile_pool(name="bcp", bufs=1) as bcp, \
                 tc.tile_pool(name="yp", bufs=2) as yp:
                at_sb = [ap.tile([128, S], F32R, name=f"at{t}", tag=f"at{t}")
                         for t in range(4)]
                wp_sb = [wyp.tile([128, D], F32R, name=f"wp{t}", tag=f"wp{t}")
                        for t in range(4)]
                for t in range(4):
                    nc.sync.dma_start(wp_sb[t][:], wp_d[t * 128:(t + 1) * 128, :])

                for h in range(HG):
                    mt = h // 2
                    r0 = (h % 2) * 64
                    qT = qk_sb[mt][r0:r0 + 64, :]
                    kT = qk_sb[4 + mt][r0:r0 + 64, :]
                    for sw in range(2):            # pair of s_i chunks
                        po = [ps.tile([128, 512], F32, tag="o", bufs=4,
                                      name=f"po{h}{sw}{i}") for i in range(2)]
                        for j in range(ST16):
                            st = ps.tile([128, 1024], F32, tag="st",
                                         bufs=2, name=f"st{h}{sw}{j}")
                            for il in range(2):
                                ic = sw * 2 + il
                                nc.tensor.matmul(
                                    st[:, il * 512:(il + 1) * 512],
                                    kT[:, j * 128:(j + 1) * 128],
                                    qT[:, ic * 512:(ic + 1) * 512],
                                    start=True, stop=True)
                            pt = ptp.tile([128, 1024], F32R, tag="pt",
                                          bufs=4, name=f"pt{h}{sw}{j}")
                            nc.scalar.activation(pt[:], st[:], AF.Exp,
                                                 bias=0.0, scale=1.0)
                            for il in range(2):
                                nc.tensor.matmul(
                                    po[il][0:HD + 1, :],
                                    v_sb[j][:, h, :],
                                    pt[:, il * 512:(il + 1) * 512],
                                    start=(j == 0), stop=(j == ST16 - 1))
                        # evacuate po fast (psum banks are the scarce
                        # resource): unnormalized attn rows + Z row, then
                        # normalize in SBUF off the critical path.
                        za = np_.tile([1, 1024], F32, tag="za")
                        slots = []
                        for il in range(2):
                            ic = sw * 2 + il
                            sl = at_sb[mt][r0:r0 + 64,
                                           ic * 512:(ic + 1) * 512]
                            slots.append(sl)
                            nc.vector.tensor_copy(sl, po[il][0:64, :])
                            nc.vector.tensor_copy(
                                za[:, il * 512:(il + 1) * 512],
                                po[il][64:65, :])
                        inv = np_.tile([1, 1024], F32, tag="zb")
                        nc.vector.reciprocal(inv[:], za[:])
                        bc = bcp.tile([128, 1024], F32, tag="bc")
                        nc.gpsimd.partition_broadcast(bc[:], inv[:])
                        for il in range(2):
                            nc.vector.tensor_mul(
                                slots[il], slots[il],
                                bc[r0:r0 + 64, il * 512:(il + 1) * 512])

                # E: out = attnT.T @ wp
                for si in range(ST16):
                    py = [ps.tile([128, 512], F32, tag="o", bufs=4,
                                  name=f"py{si}{i}") for i in range(2)]
                    for t in range(4):
                        for nch in range(2):
                            nc.tensor.matmul(
                                py[nch],
                                at_sb[t][:, si * 128:(si + 1) * 128],
                                wp_sb[t][:, nch * 512:(nch + 1) * 512],
                                start=(t == 0), stop=(t == 3))
                    for nch in range(2):
                        y = yp.tile([128, 512], F32, tag="y")
                        nc.vector.tensor_copy(y[:], py[nch][:])
                        nc.sync.dma_start(
                            out_d[si * 128:(si + 1) * 128,
                                  nch * 512:(nch + 1) * 512], y[:])
    nc.compile()
    return nc


def _prep_inputs(x, w_qkv, b_qkv, w_proj):
    """Host-side shard prep: slice per head-group, fold scale, transpose x."""
    in_maps = []
    for c in range(N_CORES):
        b, g = c // 2, c % 2
        cs = g * 512
        wq = w_qkv[:, cs:cs + 512] * 0.125
        wk = w_qkv[:, 1024 + cs:1024 + cs + 512]
        wv = w_qkv[:, 2048 + cs:2048 + cs + 512]
        bq = (b_qkv[cs:cs + 512] * 0.125).reshape(4, 128).T
        in_maps.append({
            "xt": np.ascontiguousarray(x[b].T),
            "wqk": np.ascontiguousarray(np.concatenate([wq, wk], axis=1)),
            "wv": np.ascontiguousarray(wv),
            "wp": np.ascontiguousarray(w_proj[g * 512:(g + 1) * 512, :]),
            "bq": np.ascontiguousarray(bq.astype(np.float32)),
        })
    return in_maps


def kernel(x, w_qkv, b_qkv, w_proj, b_proj, _trace=False):
    x = np.asarray(x, np.float32)
    w_qkv = np.asarray(w_qkv, np.float32)
    b_qkv = np.asarray(b_qkv, np.float32)
    w_proj = np.asarray(w_proj, np.float32)
    b_proj = np.asarray(b_proj, np.float32)

    if "nc" not in _CACHE:
        _CACHE["nc"] = _build()
    nc = _CACHE["nc"]

    in_maps = _prep_inputs(x, w_qkv, b_qkv, w_proj)
    res = bass_utils.run_bass_kernel_spmd(
        nc, in_maps, core_ids=list(range(N_CORES)), trace=_trace)

    # host-side bias: b_proj plus the value-bias path through w_proj
    bias = b_proj + b_qkv[2048:3072].astype(np.float64) @ w_proj.astype(np.float64)
    bias = bias.astype(np.float32)
    out = np.empty((B, S, D), np.float32)
    for b in range(B):
        out[b] = res.results[2 * b]["out"] + res.results[2 * b + 1]["out"] + bias
    if _trace:
        return out, res
    return out


# revision 21
# speedup vs baseline: 1.5485x; 1.0425x over previous
"""Multi-head attention forward on 8 Trainium2 NeuronCores.

Sharding: core c = 2*b + g handles batch b (of 4) and head-group g (8 of 16
heads). Each core computes its group's attention output projected through its
slice of w_proj (row-parallel); the host sums the two partial products per
batch and adds the bias terms.

Math notes (exact identities, not approximations):
  - the key bias b_k adds a per-query constant to every score row, which
    softmax ignores;
  - the value bias b_v passes through attention unchanged (attn rows sum to 1)
    so its projection b_v @ w_proj is folded into the host-side bias;
  - the 1/sqrt(64) score scale is folded into w_q / b_q (exact: power of two).

Compute dtype is float32r (TensorE reduced-precision fp32 path): ~bf16 matmul
speed at 512-wide moving operands, ~2e-4 error instead of bf16's ~4e-3.

Layouts: scores are computed transposed (ST = kT.T @ qT, [s_j, s_i]) so the
exp output PT feeds the PV matmul directly as the moving operand with no
P-transpose pass. V carries an appended ones column, so the PV output row 64
is the softmax denominator Z. Normalization (1/Z) happens on the PV psum via
ACT ln->exp + GPSIMD partition-broadcast. Head pairs (2t, 2t+1) live in the
partition halves {0..63} / {64..127} of shared qk tiles, so their K=64 score
matmuls land in different PE row groups and execute concurrently.
"""

import numpy as np

import concourse.bass as bass
import concourse.tile as tile
from concourse import bacc, mybir
from concourse import bass_utils

F32 = mybir.dt.float32
F32R = mybir.dt.float32r
AF = mybir.ActivationFunctionType

B, S, D = 4, 2048, 1024
H, HD = 16, 64
HG = 8            # heads per core (group)
N_CORES = 8

_CACHE = {}


def _build():
    nc = bacc.Bacc("TRN2", target_bir_lowering=False, debug=False,
                   num_devices=N_CORES)
    xt_d = nc.dram_tensor("xt", [D, S], F32R, kind="ExternalInput").ap()
    wqk_d = nc.dram_tensor("wqk", [D, 2 * HG * HD], F32R, kind="ExternalInput").ap()
    wv_d = nc.dram_tensor("wv", [D, HG * HD], F32R, kind="ExternalInput").ap()
    wp_d = nc.dram_tensor("wp", [HG * HD, D], F32R, kind="ExternalInput").ap()
    bq_d = nc.dram_tensor("bq", [128, 4], F32, kind="ExternalInput").ap()
    out_d = nc.dram_tensor("out", [S, D], F32, kind="ExternalOutput").ap()

    KT = D // 128          # 8 k-tiles over the embedding dim
    ST16 = S // 128        # 16 tiles over sequence

    with tile.TileContext(nc) as tc:
        with (
            tc.tile_pool(name="persist", bufs=1) as pp,
            tc.tile_pool(name="psum", bufs=1, space="PSUM") as ps,
        ):
            # ---- persistent SBUF tensors ----
            qk_sb = [pp.tile([128, S], F32R, name=f"qk{m}", tag=f"qk{m}")
                     for m in range(8)]
            v_sb = [pp.tile([128, HG, HD + 1], F32R, name=f"v{j}", tag=f"v{j}")
                    for j in range(ST16)]
            bq_sb = pp.tile([128, 4], F32, tag="bq")
            nc.sync.dma_start(bq_sb[:], bq_d)
            ones_sb = pp.tile([128, HG, 1], F32, tag="ones")
            nc.vector.memset(ones_sb[:], 1.0)

            # PE clock warmup: a burst of matmul activity on garbage while
            # the input DMAs land, so phase A starts at 2.4 GHz instead of 1.2.
            wa = pp.tile([128, 128], F32, tag="wa")
            wb = pp.tile([128, 512], F32, tag="wb")
            nc.vector.memset(wa[:], 1.0)
            nc.vector.memset(wb[:], 1.0)
            wp_ps = ps.tile([128, 512], F32, tag="o", bufs=4, name="warm")
            for _ in range(8):
                nc.tensor.matmul(wp_ps[:], wa[:], wb[:], start=True, stop=True)

            # ============ phases A+B: qkT and v (xt fully resident) =========
            with tc.tile_pool(name="xtp", bufs=1) as xtp, \
                 tc.tile_pool(name="wabp", bufs=1) as wabp:
                xt_sb = [xtp.tile([128, S], F32R, name=f"xt{k}", tag=f"xt{k}")
                         for k in range(KT)]
                for k in range(KT):
                    nc.sync.dma_start(xt_sb[k][:], xt_d[k * 128:(k + 1) * 128, :])
                wv_sb = [wabp.tile([128, 512], F32R, name=f"wv{k}", tag=f"wv{k}")
                         for k in range(KT)]
                for k in range(KT):
                    nc.sync.dma_start(wv_sb[k][:], wv_d[k * 128:(k + 1) * 128, :])

                # A: qkT per m-pair (q tile mp, k tile 4+mp); wqk streamed.
                for mp in range(4):
                    wpair = wabp.tile([128, KT, 2, 128], F32R, tag="wpair", bufs=2)
                    for k in range(KT):
                        nc.sync.dma_start(
                            wpair[:, k, 0, :],
                            wqk_d[k * 128:(k + 1) * 128, mp * 128:(mp + 1) * 128])
                        nc.sync.dma_start(
                            wpair[:, k, 1, :],
                            wqk_d[k * 128:(k + 1) * 128,
                                  512 + mp * 128:512 + (mp + 1) * 128])
                    for half in range(2):          # 0 -> q tile, 1 -> k tile
                        m = mp if half == 0 else 4 + mp
                        pa = ps.tile([128, 1024], F32, tag="st", bufs=2,
                                     name=f"paA{mp}{half}")
                        pb = ps.tile([128, 1024], F32, tag="st", bufs=2,
                                     name=f"paB{mp}{half}")
                        banks = [pa[:, 0:512], pa[:, 512:1024],
                                 pb[:, 0:512], pb[:, 512:1024]]
                        for k in range(KT):
                            for n in range(4):
                                nc.tensor.matmul(
                                    banks[n],
                                    wpair[:, k, half, :],
                                    xt_sb[k][:, n * 512:(n + 1) * 512],
                                    start=(k == 0), stop=(k == KT - 1))
                        for n in range(4):
                            dst = qk_sb[m][:, n * 512:(n + 1) * 512]
                            if half == 0:
                                nc.vector.tensor_scalar_add(dst, banks[n],
                                                            bq_sb[:, mp:mp + 1])
                            else:
                                nc.vector.tensor_copy(dst, banks[n])

                # B: v natural layout, 4 accumulators in the "o" psum tag
                for sig in range(4):
                    pv4 = [ps.tile([128, 512], F32, tag="o", bufs=4,
                                   name=f"pb{sig}{i}") for i in range(4)]
                    for k in range(KT):
                        for s4 in range(4):
                            si = sig * 4 + s4
                            nc.tensor.matmul(
                                pv4[s4],
                                xt_sb[k][:, si * 128:(si + 1) * 128],
                                wv_sb[k][:],
                                start=(k == 0), stop=(k == KT - 1))
                    for s4 in range(4):
                        si = sig * 4 + s4
                        nc.vector.tensor_copy(
                            v_sb[si][:, :, 0:HD],
                            pv4[s4][:].rearrange("p (h d) -> p h d", h=HG))
                        nc.vector.tensor_copy(v_sb[si][:, :, HD:HD + 1],
                                              ones_sb[:])

            # ========= phases C+D+E: attention, normalize, projection =======
            with tc.tile_pool(name="attp", bufs=1) as ap, \
                 tc.tile_pool(name="ptp", bufs=3) as ptp, \
                 tc.tile_pool(name="wyp", bufs=1) as wyp, \
                 tc.tile_pool(name="np_", bufs=2) as np_, \
                 tc.tile_pool(name="bcp", bufs=2) as bcp, \
                 tc.tile_pool(name="yp", bufs=4) as yp:
                at_sb = [ap.tile([128, S], F32R, name=f"at{t}", tag=f"at{t}")
                         for t in range(4)]
                wp_sb = [wyp.tile([128, D], F32R, name=f"wp{t}", tag=f"wp{t}")
                        for t in range(4)]
                for t in range(4):
                    nc.sync.dma_start(wp_sb[t][:], wp_d[t * 128:(t + 1) * 128, :])

                for h in range(HG):
                    mt = h // 2
                    r0 = (h % 2) * 64
                    qT = qk_sb[mt][r0:r0 + 64, :]
                    kT = qk_sb[4 + mt][r0:r0 + 64, :]
                    for sw in range(2):            # pair of s_i chunks
                        po = [ps.tile([128, 512], F32, tag="o", bufs=4,
                                      name=f"po{h}{sw}{i}") for i in range(2)]
                        for j in range(ST16):
                            st = ps.tile([128, 1024], F32, tag="st",
                                         bufs=2, name=f"st{h}{sw}{j}")
                            for il in range(2):
                                ic = sw * 2 + il
                                nc.tensor.matmul(
                                    st[:, il * 512:(il + 1) * 512],
                                    kT[:, j * 128:(j + 1) * 128],
                                    qT[:, ic * 512:(ic + 1) * 512],
                                    start=True, stop=True)
                            pt = ptp.tile([128, 1024], F32R, tag="pt",
                                          bufs=4, name=f"pt{h}{sw}{j}")
                            nc.scalar.activation(pt[:], st[:], AF.Exp,
                                                 bias=0.0, scale=1.0)
                            for il in range(2):
                                nc.tensor.matmul(
                                    po[il][0:HD + 1, :],
                                    v_sb[j][:, h, :],
                                    pt[:, il * 512:(il + 1) * 512],
                                    start=(j == 0), stop=(j == ST16 - 1))
                        # evacuate po fast (psum banks are the scarce
                        # resource): unnormalized attn rows + Z row, then
                        # normalize in SBUF off the critical path.
                        za = np_.tile([1, 1024], F32, tag="za")
                        slots = []
                        for il in range(2):
                            ic = sw * 2 + il
                            sl = at_sb[mt][r0:r0 + 64,
                                           ic * 512:(ic + 1) * 512]
                            slots.append(sl)
                            nc.vector.tensor_copy(sl, po[il][0:64, :])
                            nc.vector.tensor_copy(
                                za[:, il * 512:(il + 1) * 512],
                                po[il][64:65, :])
                        inv = np_.tile([1, 1024], F32, tag="zb")
                        nc.vector.reciprocal(inv[:], za[:])
                        bc = bcp.tile([128, 1024], F32, tag="bc")
                        nc.gpsimd.partition_broadcast(bc[:], inv[:])
                        for il in range(2):
                            nc.vector.tensor_mul(
                                slots[il], slots[il],
                                bc[r0:r0 + 64, il * 512:(il + 1) * 512])

                # E: out = attnT.T @ wp
                for si in range(ST16):
                    py = [ps.tile([128, 512], F32, tag="o", bufs=4,
                                  name=f"py{si}{i}") for i in range(2)]
                    for t in range(4):
                        for nch in range(2):
                            nc.tensor.matmul(
                                py[nch],
                                at_sb[t][:, si * 128:(si + 1) * 128],
                                wp_sb[t][:, nch * 512:(nch + 1) * 512],
                                start=(t == 0), stop=(t == 3))
                    for nch in range(2):
                        y = yp.tile([128, 512], F32, tag="y")
                        nc.vector.tensor_copy(y[:], py[nch][:])
                        nc.sync.dma_start(
                            out_d[si * 128:(si + 1) * 128,
                                  nch * 512:(nch + 1) * 512], y[:])
    nc.compile()
    return nc


def _prep_inputs(x, w_qkv, b_qkv, w_proj):
    """Host-side shard prep: slice per head-group, fold scale, transpose x."""
    in_maps = []
    for c in range(N_CORES):
        b, g = c // 2, c % 2
        cs = g * 512
        wq = w_qkv[:, cs:cs + 512] * 0.125
        wk = w_qkv[:, 1024 + cs:1024 + cs + 512]
        wv = w_qkv[:, 2048 + cs:2048 + cs + 512]
        bq = (b_qkv[cs:cs + 512] * 0.125).reshape(4, 128).T
        in_maps.append({
            "xt": np.ascontiguousarray(x[b].T),
            "wqk": np.ascontiguousarray(np.concatenate([wq, wk], axis=1)),
            "wv": np.ascontiguousarray(wv),
            "wp": np.ascontiguousarray(w_proj[g * 512:(g + 1) * 512, :]),
            "bq": np.ascontiguousarray(bq.astype(np.float32)),
        })
    return in_maps


def kernel(x, w_qkv, b_qkv, w_proj, b_proj, _trace=False):
    x = np.asarray(x, np.float32)
    w_qkv = np.asarray(w_qkv, np.float32)
    b_qkv = np.asarray(b_qkv, np.float32)
    w_proj = np.asarray(w_proj, np.float32)
    b_proj = np.asarray(b_proj, np.float32)

    if "nc" not in _CACHE:
        _CACHE["nc"] = _build()
    nc = _CACHE["nc"]

    in_maps = _prep_inputs(x, w_qkv, b_qkv, w_proj)
    res = bass_utils.run_bass_kernel_spmd(
        nc, in_maps, core_ids=list(range(N_CORES)), trace=_trace)

    # host-side bias: b_proj plus the value-bias path through w_proj
    bias = b_proj + b_qkv[2048:3072].astype(np.float64) @ w_proj.astype(np.float64)
    bias = bias.astype(np.float32)
    out = np.empty((B, S, D), np.float32)
    for b in range(B):
        out[b] = res.results[2 * b]["out"] + res.results[2 * b + 1]["out"] + bias
    if _trace:
        return out, res
    return out


# revision 22
# speedup vs baseline: 1.5546x; 1.0039x over previous
"""Multi-head attention forward on 8 Trainium2 NeuronCores.

Sharding: core c = 2*b + g handles batch b (of 4) and head-group g (8 of 16
heads). Each core computes its group's attention output projected through its
slice of w_proj (row-parallel); the host sums the two partial products per
batch and adds the bias terms.

Math notes (exact identities, not approximations):
  - the key bias b_k adds a per-query constant to every score row, which
    softmax ignores;
  - the value bias b_v passes through attention unchanged (attn rows sum to 1)
    so its projection b_v @ w_proj is folded into the host-side bias;
  - the 1/sqrt(64) score scale is folded into w_q / b_q (exact: power of two).

Compute dtype is float32r (TensorE reduced-precision fp32 path): ~bf16 matmul
speed at 512-wide moving operands, ~2e-4 error instead of bf16's ~4e-3.

Layouts: scores are computed transposed (ST = kT.T @ qT, [s_j, s_i]) so the
exp output PT feeds the PV matmul directly as the moving operand with no
P-transpose pass. V carries an appended ones column, so the PV output row 64
is the softmax denominator Z. Normalization (1/Z) happens on the PV psum via
ACT ln->exp + GPSIMD partition-broadcast. Head pairs (2t, 2t+1) live in the
partition halves {0..63} / {64..127} of shared qk tiles, so their K=64 score
matmuls land in different PE row groups and execute concurrently.
"""

import numpy as np

import concourse.bass as bass
import concourse.tile as tile
from concourse import bacc, mybir
from concourse import bass_utils

F32 = mybir.dt.float32
F32R = mybir.dt.float32r
AF = mybir.ActivationFunctionType

B, S, D = 4, 2048, 1024
H, HD = 16, 64
HG = 8            # heads per core (group)
N_CORES = 8

_CACHE = {}


def _build():
    nc = bacc.Bacc("TRN2", target_bir_lowering=False, debug=False,
                   num_devices=N_CORES)
    xt_d = nc.dram_tensor("xt", [D, S], F32R, kind="ExternalInput").ap()
    wqk_d = nc.dram_tensor("wqk", [D, 2 * HG * HD], F32R, kind="ExternalInput").ap()
    wv_d = nc.dram_tensor("wv", [D, HG * HD], F32R, kind="ExternalInput").ap()
    wp_d = nc.dram_tensor("wp", [HG * HD, D], F32R, kind="ExternalInput").ap()
    bq_d = nc.dram_tensor("bq", [128, 4], F32, kind="ExternalInput").ap()
    out_d = nc.dram_tensor("out", [S, D], F32, kind="ExternalOutput").ap()

    KT = D // 128          # 8 k-tiles over the embedding dim
    ST16 = S // 128        # 16 tiles over sequence

    with tile.TileContext(nc) as tc:
        with (
            tc.tile_pool(name="persist", bufs=1) as pp,
            tc.tile_pool(name="psum", bufs=1, space="PSUM") as ps,
        ):
            # ---- persistent SBUF tensors ----
            qk_sb = [pp.tile([128, S], F32R, name=f"qk{m}", tag=f"qk{m}")
                     for m in range(8)]
            v_sb = [pp.tile([128, HG, HD + 1], F32R, name=f"v{j}", tag=f"v{j}")
                    for j in range(ST16)]
            bq_sb = pp.tile([128, 4], F32, tag="bq")
            nc.sync.dma_start(bq_sb[:], bq_d)
            ones_sb = pp.tile([128, HG, 1], F32, tag="ones")
            nc.vector.memset(ones_sb[:], 1.0)

            # PE clock warmup: a burst of matmul activity on garbage while
            # the input DMAs land, so phase A starts at 2.4 GHz instead of 1.2.
            wa = pp.tile([128, 128], F32, tag="wa")
            wb = pp.tile([128, 512], F32, tag="wb")
            nc.vector.memset(wa[:], 1.0)
            nc.vector.memset(wb[:], 1.0)
            wp_ps = ps.tile([128, 512], F32, tag="o", bufs=4, name="warm")
            for _ in range(26):
                nc.tensor.matmul(wp_ps[:], wa[:], wb[:], start=True, stop=True)

            # ============ phases A+B: qkT and v (xt fully resident) =========
            with tc.tile_pool(name="xtp", bufs=1) as xtp, \
                 tc.tile_pool(name="wabp", bufs=1) as wabp:
                xt_sb = [xtp.tile([128, S], F32R, name=f"xt{k}", tag=f"xt{k}")
                         for k in range(KT)]
                for k in range(KT):
                    nc.sync.dma_start(xt_sb[k][:], xt_d[k * 128:(k + 1) * 128, :])
                wv_sb = [wabp.tile([128, 512], F32R, name=f"wv{k}", tag=f"wv{k}")
                         for k in range(KT)]
                for k in range(KT):
                    nc.sync.dma_start(wv_sb[k][:], wv_d[k * 128:(k + 1) * 128, :])

                # A: qkT per m-pair (q tile mp, k tile 4+mp); wqk streamed.
                for mp in range(4):
                    wpair = wabp.tile([128, KT, 2, 128], F32R, tag="wpair", bufs=2)
                    for k in range(KT):
                        nc.sync.dma_start(
                            wpair[:, k, 0, :],
                            wqk_d[k * 128:(k + 1) * 128, mp * 128:(mp + 1) * 128])
                        nc.sync.dma_start(
                            wpair[:, k, 1, :],
                            wqk_d[k * 128:(k + 1) * 128,
                                  512 + mp * 128:512 + (mp + 1) * 128])
                    for half in range(2):          # 0 -> q tile, 1 -> k tile
                        m = mp if half == 0 else 4 + mp
                        pa = ps.tile([128, 1024], F32, tag="st", bufs=2,
                                     name=f"paA{mp}{half}")
                        pb = ps.tile([128, 1024], F32, tag="st", bufs=2,
                                     name=f"paB{mp}{half}")
                        banks = [pa[:, 0:512], pa[:, 512:1024],
                                 pb[:, 0:512], pb[:, 512:1024]]
                        for k in range(KT):
                            for n in range(4):
                                nc.tensor.matmul(
                                    banks[n],
                                    wpair[:, k, half, :],
                                    xt_sb[k][:, n * 512:(n + 1) * 512],
                                    start=(k == 0), stop=(k == KT - 1))
                        for n in range(4):
                            dst = qk_sb[m][:, n * 512:(n + 1) * 512]
                            if half == 0:
                                nc.vector.tensor_scalar_add(dst, banks[n],
                                                            bq_sb[:, mp:mp + 1])
                            else:
                                nc.vector.tensor_copy(dst, banks[n])

                # B: v natural layout, 4 accumulators in the "o" psum tag
                for sig in range(4):
                    pv4 = [ps.tile([128, 512], F32, tag="o", bufs=4,
                                   name=f"pb{sig}{i}") for i in range(4)]
                    for k in range(KT):
                        for s4 in range(4):
                            si = sig * 4 + s4
                            nc.tensor.matmul(
                                pv4[s4],
                                xt_sb[k][:, si * 128:(si + 1) * 128],
                                wv_sb[k][:],
                                start=(k == 0), stop=(k == KT - 1))
                    for s4 in range(4):
                        si = sig * 4 + s4
                        nc.vector.tensor_copy(
                            v_sb[si][:, :, 0:HD],
                            pv4[s4][:].rearrange("p (h d) -> p h d", h=HG))
                        nc.vector.tensor_copy(v_sb[si][:, :, HD:HD + 1],
                                              ones_sb[:])

            # ========= phases C+D+E: attention, normalize, projection =======
            with tc.tile_pool(name="attp", bufs=1) as ap, \
                 tc.tile_pool(name="ptp", bufs=3) as ptp, \
                 tc.tile_pool(name="wyp", bufs=1) as wyp, \
                 tc.tile_pool(name="np_", bufs=2) as np_, \
                 tc.tile_pool(name="bcp", bufs=2) as bcp, \
                 tc.tile_pool(name="yp", bufs=4) as yp:
                at_sb = [ap.tile([128, S], F32R, name=f"at{t}", tag=f"at{t}")
                         for t in range(4)]
                wp_sb = [wyp.tile([128, D], F32R, name=f"wp{t}", tag=f"wp{t}")
                        for t in range(4)]
                for t in range(4):
                    nc.sync.dma_start(wp_sb[t][:], wp_d[t * 128:(t + 1) * 128, :])

                for h in range(HG):
                    mt = h // 2
                    r0 = (h % 2) * 64
                    qT = qk_sb[mt][r0:r0 + 64, :]
                    kT = qk_sb[4 + mt][r0:r0 + 64, :]
                    for sw in range(2):            # pair of s_i chunks
                        po = [ps.tile([128, 512], F32, tag="o", bufs=4,
                                      name=f"po{h}{sw}{i}") for i in range(2)]
                        for j in range(ST16):
                            st = ps.tile([128, 1024], F32, tag="st",
                                         bufs=2, name=f"st{h}{sw}{j}")
                            for il in range(2):
                                ic = sw * 2 + il
                                nc.tensor.matmul(
                                    st[:, il * 512:(il + 1) * 512],
                                    kT[:, j * 128:(j + 1) * 128],
                                    qT[:, ic * 512:(ic + 1) * 512],
                                    start=True, stop=True)
                            pt = ptp.tile([128, 1024], F32R, tag="pt",
                                          bufs=4, name=f"pt{h}{sw}{j}")
                            nc.scalar.activation(pt[:], st[:], AF.Exp,
                                                 bias=0.0, scale=1.0)
                            for il in range(2):
                                nc.tensor.matmul(
                                    po[il][0:HD + 1, :],
                                    v_sb[j][:, h, :],
                                    pt[:, il * 512:(il + 1) * 512],
                                    start=(j == 0), stop=(j == ST16 - 1))
                        # evacuate po fast (psum banks are the scarce
                        # resource): unnormalized attn rows + Z row, then
                        # normalize in SBUF off the critical path.
                        za = np_.tile([1, 1024], F32, tag="za")
                        slots = []
                        for il in range(2):
                            ic = sw * 2 + il
                            sl = at_sb[mt][r0:r0 + 64,
                                           ic * 512:(ic + 1) * 512]
                            slots.append(sl)
                            nc.vector.tensor_copy(sl, po[il][0:64, :])
                            nc.vector.tensor_copy(
                                za[:, il * 512:(il + 1) * 512],
                                po[il][64:65, :])
                        inv = np_.tile([1, 1024], F32, tag="zb")
                        nc.vector.reciprocal(inv[:], za[:])
                        bc = bcp.tile([128, 1024], F32, tag="bc")
                        nc.gpsimd.partition_broadcast(bc[:], inv[:])
                        for il in range(2):
                            nc.vector.tensor_mul(
                                slots[il], slots[il],
                                bc[r0:r0 + 64, il * 512:(il + 1) * 512])

                # E: out = attnT.T @ wp
                for si in range(ST16):
                    py = [ps.tile([128, 512], F32, tag="o", bufs=4,
                                  name=f"py{si}{i}") for i in range(2)]
                    for t in range(4):
                        for nch in range(2):
                            nc.tensor.matmul(
                                py[nch],
                                at_sb[t][:, si * 128:(si + 1) * 128],
                                wp_sb[t][:, nch * 512:(nch + 1) * 512],
                                start=(t == 0), stop=(t == 3))
                    for nch in range(2):
                        y = yp.tile([128, 512], F32, tag="y")
                        nc.vector.tensor_copy(y[:], py[nch][:])
                        nc.sync.dma_start(
                            out_d[si * 128:(si + 1) * 128,
                                  nch * 512:(nch + 1) * 512], y[:])
    nc.compile()
    return nc


def _prep_inputs(x, w_qkv, b_qkv, w_proj):
    """Host-side shard prep: slice per head-group, fold scale, transpose x."""
    in_maps = []
    for c in range(N_CORES):
        b, g = c // 2, c % 2
        cs = g * 512
        wq = w_qkv[:, cs:cs + 512] * 0.125
        wk = w_qkv[:, 1024 + cs:1024 + cs + 512]
        wv = w_qkv[:, 2048 + cs:2048 + cs + 512]
        bq = (b_qkv[cs:cs + 512] * 0.125).reshape(4, 128).T
        in_maps.append({
            "xt": np.ascontiguousarray(x[b].T),
            "wqk": np.ascontiguousarray(np.concatenate([wq, wk], axis=1)),
            "wv": np.ascontiguousarray(wv),
            "wp": np.ascontiguousarray(w_proj[g * 512:(g + 1) * 512, :]),
            "bq": np.ascontiguousarray(bq.astype(np.float32)),
        })
    return in_maps


def kernel(x, w_qkv, b_qkv, w_proj, b_proj, _trace=False):
    x = np.asarray(x, np.float32)
    w_qkv = np.asarray(w_qkv, np.float32)
    b_qkv = np.asarray(b_qkv, np.float32)
    w_proj = np.asarray(w_proj, np.float32)
    b_proj = np.asarray(b_proj, np.float32)

    if "nc" not in _CACHE:
        _CACHE["nc"] = _build()
    nc = _CACHE["nc"]

    in_maps = _prep_inputs(x, w_qkv, b_qkv, w_proj)
    res = bass_utils.run_bass_kernel_spmd(
        nc, in_maps, core_ids=list(range(N_CORES)), trace=_trace)

    # host-side bias: b_proj plus the value-bias path through w_proj
    bias = b_proj + b_qkv[2048:3072].astype(np.float64) @ w_proj.astype(np.float64)
    bias = bias.astype(np.float32)
    out = np.empty((B, S, D), np.float32)
    for b in range(B):
        out[b] = res.results[2 * b]["out"] + res.results[2 * b + 1]["out"] + bias
    if _trace:
        return out, res
    return out
